# revision 2
# baseline (speedup 1.0000x reference)
"""Correlation cost-volume kernel for Trainium2 (Bass/Tile), data-parallel
over batch across 8 NeuronCores.

Math: cost[b, i, h, j] = mean_c(left[b, :, h, j] * right[b, :, h, j - i])
for j >= i else 0, with i in [0, 64).

Per (b, h) this is a 64-diagonal band of the Gram matrix M' = L^T R where
L/R are [C=128, W=512] slices. Each band tile t (j in [128t, 128t+128))
is one PE matmul: lhsT = L[:, jblock] (stationary), rhs = a 191-column
window of R (shifted by -63, zero-padded on the left), giving PSUM tile
P_t[p, x] = M'[128t + p, 128t - 63 + x].

The output needs P_t[p, p + k] (k = 63 - i) — a skewed (diagonal-band)
read. On-chip SBUF/PSUM access patterns cannot step across partitions with
a byte remainder (HWDGE crashes, SWDGE wraps the offset mod 16B — both
verified on hardware), so the band tiles take a round trip through a DRAM
scratch where the address space is flat and the skew is a plain strided
access pattern: element (t, p, k) of the band sits at scratch offset
p*(NT*TW+1) + t*TW + k.  The device writes OUT[h, j, k] with k contiguous;
the host flips k -> i = 63-k and transposes to [b, i, h, j].
"""

import numpy as np

import concourse.bass as bass
import concourse.mybir as mybir
import concourse.tile as _tile
from concourse.bass_types import AP
from concourse.tile import TileContext
from concourse.vector_clock import ScopedClock
from concourse import bass_utils

F32 = mybir.dt.float32

B = 8     # batch == number of cores
C = 128   # channels (contraction dim)
H = 160   # rows
W = 512   # width
D = 64    # displacements
TB = 128  # j-block width (matmul M)
NT = W // TB          # 4 band tiles per row
NW = TB + D - 1       # 191-column rhs window per tile
TW = 192              # scratch tile column pitch (>= NW)
RP = D + W            # padded right row width (64 zeros + 512)
HB = 8                # h rows loaded per input DMA


# ---------------------------------------------------------------------------
# Workarounds: the walrus build in this container rejects instructions that
# carry more than one semaphore sync-wait. Split extra waits onto preceding
# single-wait instructions.
# ---------------------------------------------------------------------------

def _patched_drain_and_barrier(self, tick_clock, wait_clock):
    drain_inst = self.nc.sync.drain()
    wait_clock.add_sem_waits(
        drain_inst.ins, ScopedClock({None: tick_clock.global_clock})
    )
    si = drain_inst.ins.sync_info
    if si is not None and si.on_wait and len(si.on_wait) > 1:
        waits = list(si.on_wait)
        drain_inst.ins.sync_info = mybir.SyncInfo(
            on_wait=[waits[0]], on_update=list(si.on_update or [])
        )
        for w in waits[1:]:
            d2 = self.nc.sync.drain()
            d2.ins.sync_info = mybir.SyncInfo(on_wait=[w], on_update=[])
    self.nc.all_engine_barrier()
    assert self.sems is not None
    popped = self.nc._tile_sem_poison_stack.pop()
    assert popped is self._sem_poison
    self.nc.clear_and_free_semaphores(list(self.sems.allocated().values()))
    self.nc.all_engine_barrier()


_tile.TileContext._drain_and_barrier = _patched_drain_and_barrier

_split_counter = [0]


def _split_multiwaits(nc):
    for fn in nc.m.functions:
        for bb in fn.blocks:
            insts = list(bb.instructions)
            out = []
            changed = False
            for inst in insts:
                si = inst.sync_info
                if si is not None and si.on_wait and len(si.on_wait) > 1:
                    waits = list(si.on_wait)
                    for w in waits[:-1]:
                        _split_counter[0] += 1
                        nop = mybir.InstNoOp(
                            name=f"wsplit{_split_counter[0]}",
                            engine=inst.engine,
                            ins=[],
                            outs=[],
                            sync_info=mybir.SyncInfo(on_wait=[w], on_update=[]),
                        )
                        out.append(nop)
                    inst.sync_info = mybir.SyncInfo(
                        on_wait=[waits[-1]], on_update=list(si.on_update or [])
                    )
                    changed = True
                out.append(inst)
            if changed:
                bb.instructions[:] = out


# ---------------------------------------------------------------------------
# Kernel program (identical on every core; each core gets one batch element)
# ---------------------------------------------------------------------------

def _build():
    nc = bass.Bass()
    Lt = nc.dram_tensor("left", [C, H, W], F32, kind="ExternalInput")
    Rt = nc.dram_tensor("right", [C, H, W], F32, kind="ExternalInput")
    OUT = nc.dram_tensor("out", [H, W, D], F32, kind="ExternalOutput")

    with TileContext(nc) as tc:
        with (
            tc.tile_pool(name="io", bufs=2) as io_pool,
            tc.tile_pool(name="sp", bufs=3) as s_pool,
            tc.tile_pool(name="ps", bufs=8, space="PSUM") as ps_pool,
            tc.tile_pool(name="dr", bufs=4, space="DRAM") as dr_pool,
        ):
            for hb in range(H // HB):
                l_blk = io_pool.tile([C, HB * W], F32, name="l_blk")
                r_blk = io_pool.tile([C, HB * RP], F32, name="r_blk")
                rrow = r_blk.ap[0][0]
                nc.sync.dma_start(out=l_blk, in_=Lt[:, hb * HB : (hb + 1) * HB, :])
                nc.vector.memset(
                    AP(r_blk.tensor, r_blk.offset, [[rrow, C], [RP, HB], [1, D]]),
                    0.0,
                )
                nc.sync.dma_start(
                    out=AP(
                        r_blk.tensor, r_blk.offset + D, [[rrow, C], [RP, HB], [1, W]]
                    ),
                    in_=Rt[:, hb * HB : (hb + 1) * HB, :],
                )
                for hh in range(HB):
                    h = hb * HB + hh
                    s_t = s_pool.tile([C, NT * TW], F32, name="s_t")
                    for t in range(NT):
                        psum = ps_pool.tile([C, 192], F32, name="psum", tag="psum")
                        lhsT = l_blk[:, hh * W + TB * t : hh * W + TB * t + TB]
                        rhs = r_blk[
                            :, hh * RP + TB * t + 1 : hh * RP + TB * t + 1 + NW
                        ]
                        nc.tensor.matmul(
                            psum[:, 0:NW], lhsT, rhs, start=True, stop=True
                        )
                        nc.vector.tensor_scalar_mul(
                            s_t[:, TW * t : TW * t + NW], psum[:, 0:NW], 1.0 / C
                        )
                    scratch = dr_pool.tile([C, NT * TW], F32, name="scratch")
                    nc.sync.dma_start(out=scratch, in_=s_t)
                    skew_in = AP(
                        scratch.tensor,
                        scratch.offset,
                        [[TW, NT], [NT * TW + 1, C], [1, D]],
                    )
                    out_ap = AP(OUT, h * W * D, [[TB * D, NT], [D, TB], [1, D]])
                    nc.scalar.dma_start(out=out_ap, in_=skew_in)

    _split_multiwaits(nc)
    return nc


_NC_CACHE = []


def kernel(left_feature: np.ndarray, right_feature: np.ndarray) -> np.ndarray:
    left_feature = np.asarray(left_feature, dtype=np.float32)
    right_feature = np.asarray(right_feature, dtype=np.float32)
    assert left_feature.shape == (B, C, H, W), left_feature.shape
    assert right_feature.shape == (B, C, H, W), right_feature.shape
    if not _NC_CACHE:
        _NC_CACHE.append(_build())
    nc = _NC_CACHE[0]
    in_maps = [
        {
            "left": np.ascontiguousarray(left_feature[b]),
            "right": np.ascontiguousarray(right_feature[b]),
        }
        for b in range(B)
    ]
    r = bass_utils.run_bass_kernel_spmd(nc, in_maps, core_ids=list(range(B)))
    outs = np.stack([res["out"] for res in r.results])  # [B, H, W, D], k = 63 - i
    return np.ascontiguousarray(np.flip(outs, axis=3).transpose(0, 3, 1, 2))
